# revision 3
# baseline (speedup 1.0000x reference)
"""Trainium2 Bass kernel for the ConvModule problem — Winograd F(4,5).

Per core (2 batches, data-parallel across 8 cores):
    LayerNorm -> pw conv C->2C (+Swish) -> k=5 conv 2C->2C  -> GLU
    -> BatchNorm (folded into conv3) -> pw conv C->C

The k=5 conv is 89% of PE work; Winograd F(4,5) computes 4 outputs from 8
tap-products instead of 20 (2.5x fewer PE cycles).  Winograd matmuls run in
float32r (1 cycle/row at >=256 moving columns, ~13-bit effective mantissa);
everything else in bf16.  The time-domain transforms run on the Vector
engine (stride-4 window reads of the fp16 Swish output); the inverse
transform is folded into S/D pair combinations accumulated in fp16.
Winograd weights (32 MB float32r) are streamed from DRAM per tap point,
overlapped with the matmuls of the previous tap pair.
"""

import numpy as np
from contextlib import ExitStack

import concourse.bass as bass
import concourse.bacc as bacc
import concourse.tile as tile
from concourse import mybir
from concourse.masks import make_identity
from concourse.bass_utils import run_bass_kernel_spmd

B, T, C, K = 16, 1024, 512, 5
EPS_LN = 1e-5
EPS_BN = 1e-5
NCORES = 8
BLOC = B // NCORES
P = 128
CB = C // P                 # 4
OB = (2 * C) // P           # 8
M = 4                       # Winograd outputs per tile
N = M + K - 1               # 8 tap points
NT = T // M                 # 256 tau per batch
NT2 = BLOC * NT             # 512 columns (both batches fused)
F32 = mybir.dt.float32
F32R = mybir.dt.float32r
BF16 = mybir.dt.bfloat16
F16 = mybir.dt.float16
AF = mybir.ActivationFunctionType
OP = mybir.AluOpType

POINTS = [0.0, 1.0, -1.0, 2.0, -2.0, 0.5, -0.5]
PAIR_ROWS = [(3, 4), (1, 2), (5, 6), (0, 7)]  # BT/AT row pairs per step


def build_winograd_matrices():
    """AT (4x8), G (8x5), BT (8x8) for correlation F(4,5)."""
    rs = np.random.RandomState(0)
    n, m, r = N, M, K
    AT = np.zeros((m, n))
    for j, p in enumerate(POINTS):
        AT[:, j] = [p**v for v in range(m)]
    AT[m - 1, n - 1] = 1.0
    G = np.zeros((n, r))
    for i, p in enumerate(POINTS):
        G[i] = [p**k for k in range(r)]
    G[n - 1, r - 1] = 1.0
    S = 400
    D = rs.randn(S, n)
    Gm = rs.randn(S, r)
    rows, rhs = [], []
    for s in range(S):
        gt = G @ Gm[s]
        for v in range(m):
            rows.append((AT[v][:, None] * gt[:, None] * D[s][None, :]).ravel())
            rhs.append(np.dot(D[s, v : v + r], Gm[s]))
    BT, *_ = np.linalg.lstsq(np.array(rows), np.array(rhs), rcond=None)
    BT = BT.reshape(n, n)
    rs2 = np.random.RandomState(5)
    Dv, Gv = rs2.randn(20, n), rs2.randn(20, r)
    for s in range(20):
        ref = np.array([np.dot(Dv[s, v : v + r], Gv[s]) for v in range(m)])
        got = AT @ ((G @ Gv[s]) * (BT @ Dv[s]))
        assert np.abs(got - ref).max() < 1e-9 * max(1.0, np.abs(ref).max())
    BT[np.abs(BT) < 1e-9] = 0.0
    return AT, G, BT


AT_W, G_W, BT_W = build_winograd_matrices()

# Per-row scales so each pair's even-chain lead coefficient is exactly 1
# (compensated in the transformed weights): d~_j' = s_j d~_j, w~_j' = w~_j/s_j.
SCALE_W = np.ones(N)
for _jp, _jm in [(1, 2), (3, 4), (5, 6)]:
    SCALE_W[_jp] = SCALE_W[_jm] = 1.0 / BT_W[_jp][2]
BT_N = BT_W * SCALE_W[:, None]


def build_nc() -> bass.Bass:
    nc = bacc.Bacc("TRN2")

    xs = nc.declare_dram_parameter("xs", [BLOC, T, C], F32, isOutput=False)
    w1t = nc.declare_dram_parameter("w1t", [CB, P, 2 * C], BF16, isOutput=False)
    # Winograd weights: [j, ob, p(in-chan within ib), ib, out-col] so a
    # (j, ob) slice is one contiguous DMA with contiguous per-partition rows.
    w2w = nc.declare_dram_parameter("w2w", [N, OB, P, OB, P], F32R, isOutput=False)
    w3t = nc.declare_dram_parameter("w3t", [CB, P, C], BF16, isOutput=False)
    b1 = nc.declare_dram_parameter("b1", [P, OB], F32, isOutput=False)
    b2v = nc.declare_dram_parameter("b2v", [P, CB], F32, isOutput=False)
    b2g = nc.declare_dram_parameter("b2g", [P, CB], F32, isOutput=False)
    b3r = nc.declare_dram_parameter("b3r", [1, C], BF16, isOutput=False)
    out = nc.declare_dram_parameter("out", [BLOC, T, C], F32, isOutput=True)

    with ExitStack() as ctx:
        tc = ctx.enter_context(tile.TileContext(nc))

        consts = ctx.enter_context(tc.tile_pool(name="consts", bufs=1))
        xinp = ctx.enter_context(tc.tile_pool(name="xinp", bufs=3))
        stats = ctx.enter_context(tc.tile_pool(name="stats", bufs=4))
        w2pool = ctx.enter_context(tc.tile_pool(name="w2pool", bufs=2))
        dtpA = ctx.enter_context(tc.tile_pool(name="dtpA", bufs=2))
        dtpB = ctx.enter_context(tc.tile_pool(name="dtpB", bufs=1))
        tmpp = ctx.enter_context(tc.tile_pool(name="tmpp", bufs=2))
        invp = ctx.enter_context(tc.tile_pool(name="invp", bufs=2))
        outp = ctx.enter_context(tc.tile_pool(name="outp", bufs=2))
        tp_psum = ctx.enter_context(tc.tile_pool(name="tp_psum", bufs=2, space="PSUM"))
        mm_psum = ctx.enter_context(tc.tile_pool(name="mm_psum", bufs=2, space="PSUM"))
        wpsA = ctx.enter_context(tc.tile_pool(name="wpsA", bufs=2, space="PSUM"))
        wpsB = ctx.enter_context(tc.tile_pool(name="wpsB", bufs=2, space="PSUM"))

        # ---- constants / small weights ----
        ident = consts.tile([P, P], BF16, tag="ident")
        make_identity(nc, ident)
        epssb = consts.tile([P, 1], F32, tag="eps")
        nc.vector.memset(epssb, EPS_LN)
        epssb2 = consts.tile([P, 1], F32, tag="eps2")
        nc.vector.memset(epssb2, EPS_LN * C * C)
        onesb = consts.tile([1, P], BF16, tag="ones")
        nc.vector.memset(onesb, 1.0)
        b1sb = consts.tile([P, OB], F32, tag="b1")
        nc.sync.dma_start(out=b1sb, in_=b1[:])
        b2vsb = consts.tile([P, CB], F32, tag="b2v")
        nc.sync.dma_start(out=b2vsb, in_=b2v[:])
        b2gsb = consts.tile([P, CB], F32, tag="b2g")
        nc.sync.dma_start(out=b2gsb, in_=b2g[:])
        b3sb = consts.tile([1, C], BF16, tag="b3")
        nc.sync.dma_start(out=b3sb, in_=b3r[:])
        w1sb = []
        for cb in range(CB):
            w = consts.tile([P, 2 * C], BF16, tag=f"w1_{cb}", name=f"w1_{cb}")
            nc.sync.dma_start(out=w, in_=w1t[cb])
            w1sb.append(w)
        w3sb = []
        for cb in range(CB):
            w = consts.tile([P, C], BF16, tag=f"w3_{cb}", name=f"w3_{cb}")
            nc.sync.dma_start(out=w, in_=w3t[cb])
            w3sb.append(w)

        # ---- persistent activations ----
        # h1[ib]: Swish output fp16, col = t + 2 (2 pad left, 2 pad right,
        # padded to 1032 for the stride-4 rearrange)
        HCOLS = 1032
        h1 = []
        for g in range(OB // 2):
            t_ = consts.tile([P, 2, BLOC, HCOLS], F16, tag=f"h1_{g}", name=f"h1_{g}")
            nc.vector.memset(t_[:, :, :, 0:2], 0.0)
            nc.vector.memset(t_[:, :, :, T + 2 : T + 6], 0.0)
            h1.append(t_)
        hN = []
        for b in range(BLOC):
            t_ = consts.tile([P, CB, T], BF16, tag=f"hN_{b}", name=f"hN_{b}")
            hN.append(t_)
        # vacc[v][ob]: [P, b, tau] fp16 inverse-transform accumulators
        vacc = [
            [consts.tile([P, BLOC, NT], F16, tag=f"va{v}_{ob}", name=f"va{v}_{ob}")
             for ob in range(OB)]
            for v in range(M)
        ]
        # hG[cb]: GLU output bf16 (conv3 stationary), v-major layout
        # [P, b, v, tau]; the output DMA unpermutes rows back to t = 4*tau+v.
        hG = [
            consts.tile([P, BLOC, T], BF16, tag=f"hG_{cb}", name=f"hG_{cb}")
            for cb in range(CB)
        ]

        # ================= Phase A (both batches), then B interleaved =====
        for b in range(BLOC):
            hN3 = hN[b]
            for tb in range(T // P):
                xt = xinp.tile([P, C], F32, tag="xch")
                nc.gpsimd.dma_start(out=xt, in_=xs[b, tb * P : (tb + 1) * P])
                st6 = stats.tile([P, 6], F32, tag="st6")
                nc.vector.bn_stats(out=st6, in_=xt)
                mv = stats.tile([P, 2], F32, tag="mv")
                nc.vector.bn_aggr(out=mv, in_=st6)
                rstd = stats.tile([P, 1], F32, tag="rstd")
                nc.scalar.activation(
                    out=rstd, in_=mv[:, 1:2], func=AF.Sqrt, bias=epssb, scale=1.0
                )
                nc.vector.reciprocal(out=rstd, in_=rstd)
                xn = xinp.tile([P, C], BF16, tag="xn")
                nc.vector.tensor_scalar(
                    out=xn, in0=xt, scalar1=mv[:, 0:1], scalar2=rstd,
                    op0=OP.subtract, op1=OP.mult,
                )
                ps = tp_psum.tile([P, CB * P], BF16, tag="tp")
                for cb in range(CB):
                    nc.tensor.transpose(
                        ps[:, cb * P : (cb + 1) * P],
                        xn[:, cb * P : (cb + 1) * P],
                        ident,
                    )
                nc.scalar.copy(
                    out=hN3[:, :, tb * P : (tb + 1) * P],
                    in_=ps[:, :].rearrange("p (c i) -> p c i", c=CB),
                )

        # ---- conv1 + Swish(psum + b1) -> h1 fp16; ob-major so transforms
        # (which need both batches of an ib block) can start early ----
        for ob in range(OB):
            for b in range(BLOC):
                for h in range(2):
                    pA = mm_psum.tile([P, 512], F32, tag="mm512")
                    for cb in range(CB):
                        nc.tensor.matmul(
                            pA,
                            w1sb[cb][:, ob * P : (ob + 1) * P],
                            hN[b][:, cb, h * 512 : (h + 1) * 512],
                            start=(cb == 0), stop=(cb == CB - 1),
                        )
                    nc.scalar.activation(
                        out=h1[ob // 2][:, ob % 2, b, 2 + h * 512 : 2 + (h + 1) * 512],
                        in_=pA,
                        func=AF.Silu,
                        bias=b1sb[:, ob : ob + 1],
                        scale=1.0,
                    )

        # ================= Winograd =================
        def win_view(g, o):
            """[P, 2, BLOC, NT] fused ib-pair view of col 4*tau+o."""
            hv = h1[g][:, :, :, :].rearrange(
                "p i b (tau four) -> p i b tau four", four=4
            )
            return hv[:, :, :, o // 4 : o // 4 + NT, o % 4]

        def chain(eng, out_tile, ib, terms):
            """out = sum (c * win(o)) for (o, c) in terms.  If the first
            coefficient is 1.0 the head op is an STT with the unit-coeff
            window as in1 (skips the TS head)."""
            if len(terms) >= 2 and abs(terms[0][1] - 1.0) < 1e-9:
                (o0, _), (o1, c1) = terms[0], terms[1]
                eng.scalar_tensor_tensor(
                    out=out_tile, in0=win_view(ib, o1), scalar=float(c1),
                    in1=win_view(ib, o0), op0=OP.mult, op1=OP.add,
                )
                rest = terms[2:]
            else:
                eng.tensor_scalar(
                    out=out_tile, in0=win_view(ib, terms[0][0]),
                    scalar1=float(terms[0][1]), scalar2=None, op0=OP.mult,
                )
                rest = terms[1:]
            for o, c in rest:
                eng.scalar_tensor_tensor(
                    out=out_tile, in0=win_view(ib, o), scalar=float(c),
                    in1=out_tile, op0=OP.mult, op1=OP.add,
                )

        def emit_transform(pi):
            """Emit fused data-transform ops for pair pi; returns
            {(g, j): tile [P, 2, BLOC, NT]}."""
            jp, jm = PAIR_ROWS[pi]
            res = {}
            for g in range(OB // 2):
                eng = nc.vector
                if pi < 3:
                    tE = tmpp.tile([P, 2, BLOC, NT], F16, tag="tE", name=f"tE_{pi}_{g}")
                    chain(eng, tE, g, [(o, BT_N[jp][o]) for o in (2, 4, 6)])
                    tO = tmpp.tile([P, 2, BLOC, NT], F16, tag="tO", name=f"tO_{pi}_{g}")
                    chain(eng, tO, g, [(o, BT_N[jp][o]) for o in (1, 3, 5)])
                    dp = dtpA.tile([P, 2, BLOC, NT], F32R, tag=f"dt{g}a",
                                   name=f"dp_{pi}_{g}")
                    dm = dtpB.tile([P, 2, BLOC, NT], F32R, tag=f"dt{g}b",
                                   name=f"dm_{pi}_{g}")
                    eng.tensor_add(out=dp, in0=tE, in1=tO)
                    eng.tensor_tensor(out=dm, in0=tE, in1=tO, op=OP.subtract)
                    res[(g, jp)] = dp
                    res[(g, jm)] = dm
                else:
                    row_terms = {
                        jp: [(0, 1.0), (2, BT_N[jp][2]), (4, BT_N[jp][4]),
                             (6, BT_N[jp][6])],
                        jm: [(7, 1.0), (1, BT_N[jm][1]), (3, BT_N[jm][3]),
                             (5, BT_N[jm][5])],
                    }
                    for tag_sfx, j in (("a", jp), ("b", jm)):
                        pool_ = dtpA if tag_sfx == "a" else dtpB
                        d = pool_.tile([P, 2, BLOC, NT], F32R, tag=f"dt{g}{tag_sfx}",
                                       name=f"d_{pi}_{g}_{j}")
                        chain(eng, d, g, row_terms[j])
                        res[(g, j)] = d
            return res

        def emit_glu(cb):
            hGv = hG[cb][:, :, :].rearrange("p b (tau v) -> p b tau v", v=M)
            for v in range(M):
                sg = invp.tile([P, BLOC, NT], F16, tag="sg", name=f"sg_{cb}_{v}")
                nc.scalar.activation(
                    out=sg, in_=vacc[v][4 + cb][:, :, :],
                    func=AF.Sigmoid, bias=b2gsb[:, cb : cb + 1], scale=1.0,
                )
                for b in range(BLOC):
                    nc.vector.scalar_tensor_tensor(
                        out=hGv[:, b, :, v],
                        in0=vacc[v][cb][:, b, :],
                        scalar=b2vsb[:, cb : cb + 1],
                        in1=sg[:, b, :],
                        op0=OP.add, op1=OP.mult,
                    )

        dts = emit_transform(0)
        for pi in range(len(PAIR_ROWS)):
            jp, jm = PAIR_ROWS[pi]
            # AT fold coefficients for this pair via S/D decomposition
            cs = (AT_W[:, jp] + AT_W[:, jm]) / 2.0
            cd = (AT_W[:, jp] - AT_W[:, jm]) / 2.0
            # weight stream + matmuls + folds per ob; on the last pair do
            # gate blocks first so GLU can fire as value blocks complete
            last = pi == len(PAIR_ROWS) - 1
            ob_order = [4, 0, 5, 1, 6, 2, 7, 3] if last else list(range(OB))
            for obi, ob in enumerate(ob_order):
                wjp = w2pool.tile([P, OB, P], F32R, tag="wA", name=f"wA_{pi}_{ob}")
                nc.sync.dma_start(out=wjp, in_=w2w[jp, ob])
                wjm = w2pool.tile([P, OB, P], F32R, tag="wB", name=f"wB_{pi}_{ob}")
                nc.sync.dma_start(out=wjm, in_=w2w[jm, ob])
                mp = wpsA.tile([P, NT2], F32, tag="mp")
                mm = wpsB.tile([P, NT2], F32, tag="mm")
                for ib in range(OB):
                    dtl = dts[(ib // 2, jp)][:, ib % 2, :, :].rearrange(
                        "p b t -> p (b t)")
                    nc.tensor.matmul(
                        mp, wjp[:, ib, :], dtl,
                        start=(ib == 0), stop=(ib == OB - 1),
                    )
                for ib in range(OB):
                    dtl = dts[(ib // 2, jm)][:, ib % 2, :, :].rearrange(
                        "p b t -> p (b t)")
                    nc.tensor.matmul(
                        mm, wjm[:, ib, :], dtl,
                        start=(ib == 0), stop=(ib == OB - 1),
                    )
                # emit next pair's transforms between this pair's matmuls and
                # folds (once, at ob==1) so DVE has independent work queued
                if obi == 0 and pi + 1 < len(PAIR_ROWS):
                    next_dts = emit_transform(pi + 1)
                # fold into vacc
                cp = invp.tile([P, NT2], F16, tag="cp")
                nc.scalar.copy(out=cp, in_=mp)
                cm = invp.tile([P, NT2], F16, tag="cm")
                nc.scalar.copy(out=cm, in_=mm)
                if pi < 3:
                    Sv = invp.tile([P, NT2], F16, tag="Sv")
                    nc.vector.tensor_add(out=Sv, in0=cp, in1=cm)
                    Dv = invp.tile([P, NT2], F16, tag="Dv")
                    nc.vector.tensor_tensor(
                        out=Dv, in0=cp, in1=cm, op=OP.subtract
                    )
                    for v in range(M):
                        src, c = (Sv, cs[v]) if v % 2 == 0 else (Dv, cd[v])
                        if c == 0.0:
                            continue
                        vt = vacc[v][ob][:, :, :].rearrange("p b t -> p (b t)")
                        if pi == 0:
                            nc.vector.tensor_scalar(
                                out=vt, in0=src, scalar1=float(c), scalar2=None,
                                op0=OP.mult,
                            )
                        elif c == 1.0:
                            nc.vector.tensor_add(out=vt, in0=src, in1=vt)
                        else:
                            nc.vector.scalar_tensor_tensor(
                                out=vt, in0=src, scalar=float(c), in1=vt,
                                op0=OP.mult, op1=OP.add,
                            )
                else:
                    # point 0 -> v0 += cp ; inf -> v3 += cm
                    v0 = vacc[0][ob][:, :, :].rearrange("p b t -> p (b t)")
                    nc.vector.tensor_add(out=v0, in0=cp, in1=v0)
                    v3 = vacc[3][ob][:, :, :].rearrange("p b t -> p (b t)")
                    nc.vector.tensor_add(out=v3, in0=cm, in1=v3)
                if last and ob < CB:
                    emit_glu(ob)
            if pi + 1 < len(PAIR_ROWS):
                dts = next_dts

        # ================= conv3 (+b3 via ones-row matmul) =================
        for b in range(BLOC):
            for tb in range(T // P):
                po = mm_psum.tile([P, 512], F32, tag="mm512")
                for cb in range(CB):
                    nc.tensor.matmul(
                        po,
                        hG[cb][:, b, tb * P : (tb + 1) * P],
                        w3sb[cb],
                        start=(cb == 0), stop=False,
                    )
                nc.tensor.matmul(po, onesb, b3sb, start=False, stop=True)
                ob_ = outp.tile([P, C], F32, tag="obig")
                nc.scalar.copy(out=ob_, in_=po)
                nc.gpsimd.dma_start(out=out[b, tb * P : (tb + 1) * P], in_=ob_)

    nc.compile()
    return nc


def prepare_inputs(x, ln_g, ln_b, w1, b1, w2, b2, bn_g, bn_b, bn_mean, bn_var, w3, b3):
    f = np.float32
    x = np.asarray(x, f)
    ln_g, ln_b = np.asarray(ln_g, f), np.asarray(ln_b, f)
    w1, b1 = np.asarray(w1, f), np.asarray(b1, f)
    w2, b2 = np.asarray(w2, f), np.asarray(b2, f)
    bn_g, bn_b = np.asarray(bn_g, f), np.asarray(bn_b, f)
    bn_mean, bn_var = np.asarray(bn_mean, f), np.asarray(bn_var, f)
    w3, b3 = np.asarray(w3, f), np.asarray(b3, f)

    w1f = w1 * ln_g[None, :]
    b1f = b1 + w1 @ ln_b
    s_bn = bn_g / np.sqrt(bn_var + EPS_BN)
    w3f = w3 * s_bn[None, :]
    b3f = b3 + w3 @ (bn_b - bn_mean * s_bn)

    # Winograd weight transform, fp64 host-side:
    # wt[j, i, o] = sum_u G[j, u] w2[u, i, o]
    wt = np.einsum("ju,uio->jio", G_W, w2.astype(np.float64))
    wt = (wt / SCALE_W[:, None, None]).astype(f)
    # device layout [j, ob, p, ib, c] = wt[j, ib*P + p ... ] with
    # in-channel i = ib*P + p, out-channel o = ob*P + c
    wt5 = wt.reshape(N, OB, P, OB, P)              # [j, ib, p, ob, c]
    w2d = np.ascontiguousarray(wt5.transpose(0, 3, 2, 1, 4))  # [j, ob, p, ib, c]

    bf = mybir.dt.np(BF16)
    w1d = np.ascontiguousarray(w1f.T.reshape(CB, P, 2 * C)).astype(bf)
    w3d = np.ascontiguousarray(w3f.T.reshape(CB, P, C)).astype(bf)
    b1d = np.ascontiguousarray(b1f.reshape(OB, P).T)
    b2vd = np.ascontiguousarray(b2[:C].reshape(CB, P).T)
    b2gd = np.ascontiguousarray(b2[C:].reshape(CB, P).T)
    b3d = np.ascontiguousarray(b3f.reshape(1, C)).astype(bf)

    shared = {"w1t": w1d, "w2w": w2d, "w3t": w3d, "b1": b1d,
              "b2v": b2vd, "b2g": b2gd, "b3r": b3d}
    in_maps = []
    for c in range(NCORES):
        m = dict(shared)
        m["xs"] = np.ascontiguousarray(x[c * BLOC : (c + 1) * BLOC])
        in_maps.append(m)
    return in_maps


_NC = None
LAST_RESULTS = None


def kernel(**inputs) -> np.ndarray:
    global _NC, LAST_RESULTS
    if _NC is None:
        _NC = build_nc()
    in_maps = prepare_inputs(**inputs)
    res = run_bass_kernel_spmd(_NC, in_maps, list(range(NCORES)))
    LAST_RESULTS = res
    return np.concatenate([r["out"] for r in res.results], axis=0)
